# revision 1
# baseline (speedup 1.0000x reference)
"""AWQ 4-bit quantized linear (group size 128) on 8 Trainium2 NeuronCores.

Column-parallel: each core owns OUT/8 = 1376 output columns. The host does
layout-only prep (slicing, int4->uint8 nibble widening with the AWQ column
permutation, dtype widening of the 4-bit zeros to fp16, transposes); all
arithmetic — zero-point subtract, scale multiply, matmul, bias — runs on
device.

Per-core device pipeline, for each 128-row block of output columns (o-tile):
  1. DMA the packed-weight rows (uint8 nibbles, o on partitions).
  2. DVE/ACT dequant: w = nib * s[o] - (z*s)[o]  (fp16 out), per k-group.
  3. DMA xbar transpose each 8-group chunk to [k=128, g, o].
  4. PE matmul accumulation over the 32 k-groups into PSUM.
  5. ACT evacuation: out = Identity(psum + bias[o]) -> fp16, DMA to DRAM.

Schedule (v3): the front is restructured so the PE rides the replicated-x
HBM stream instead of waiting for it, while keeping the DMA count low
(too many early DMAs exhaust the 8 rotating DMA semaphore lanes and
serialize unrelated transfers):
  - ~12 warmup matmuls on a zeroed scratch tile run while the prologue
    DMAs land, lifting the HAM clock gate to K=8/8 before real work.
  - constants (scales / zeros / bias) are packed host-side into ONE fp16
    tensor -> one DMA instead of three, no device-side u8->f32 cast.
  - phase 1 staggers otile 1 fourteen groups behind otile 0: otile 0
    rides the x frontier while otile 1 consumes already-resident groups,
    matching the ~7us/otile dequant pipeline latency.
  - otiles 2..10 then run sequentially (2 otiles of weights prefetched).
  - the last otile runs its two m-chains back-to-back instead of
    interleaved so the first chain's ACT+store overlap the second chain.
"""

import os
import sys

import numpy as np

if "/opt/trn_rl_repo" not in sys.path:
    sys.path.insert(0, "/opt/trn_rl_repo")

M, IN, OUT = 1024, 4096, 11008
N_CORES = 8
OC = OUT // N_CORES  # 1376 output columns per core
GS = 128  # quantization group size (== matmul k-tile)
G = IN // GS  # 32 groups
PACK = 8  # int4 values per int32 word
# reference unpacks nibble k to logical column AWQ_REVERSE_ORDER.index(k);
# equivalently logical column j within a word uses shift 4*REV[j]:
REV = np.array([0, 4, 1, 5, 2, 6, 3, 7], dtype=np.uint32)

MM_N = 512  # moving-operand free size per matmul (one PSUM bank of fp32)
QG = 8  # groups per dequant/transpose chunk
WARMUP_MMS = 30
# The front is sequenced by FIFO position on the fast scalar HWDGE ring:
# the otile-0/1/2 weight triple (one DMA, 12KB rows) drains first, then
# the first three x chunks. Only the last x chunk rides the (easily
# starved) gpsimd SWDGE ring. The sync ring carries just the packed
# constants and the transposes, so the 8 rotating HWDGE semaphore lanes
# recycle fast.
X_SC = [8, 8, 8]  # x chunks on the scalar ring, behind the weight triple
X_GP = [8]  # last x chunk on the gpsimd ring

_CACHE = {}


def _unpack_int4(q: np.ndarray) -> np.ndarray:
    """[rows, cols//8] int32 -> [rows, cols] uint8 in 0..15 (AWQ order)."""
    qu = q.view(np.uint32)
    nib = (qu[:, :, None] >> (REV * 4)[None, None, :]) & 0xF
    return nib.reshape(q.shape[0], -1).astype(np.uint8)


def _build(m, k, oc, n_cores):
    import concourse.bacc as bacc
    import concourse.tile as tile
    from concourse import mybir

    F16 = mybir.dt.float16
    F32 = mybir.dt.float32
    U8 = mybir.dt.uint8

    g = k // GS
    n_otiles = (oc + 127) // 128
    n_mch = (m + MM_N - 1) // MM_N
    n_ot = n_otiles

    nc = bacc.Bacc("TRN2", target_bir_lowering=False, debug=False)
    # x pre-swizzled on host to the SBUF layout [partition, group, m] so the
    # load runs with 64 KB-contiguous per-partition descriptors
    x3 = nc.dram_tensor("x3", [128, g, m], F16, kind="ExternalInput").ap()
    # packed weights partition-major [128, otile, k]: a 2-otile load is one
    # DMA with 8KB-contiguous per-partition rows (the SDMA engines round-robin
    # rings at packet granularity, so big rows both go fast and don't get
    # starved by the concurrent 12-16KB-row x stream)
    qwP = nc.dram_tensor("qwP", [128, n_ot, k], U8, kind="ExternalInput").ap()
    # packed per-otile constants, partition-major [128, otile, 2g+1] fp16:
    # [.., 0:g] scales, [.., g:2g] zeros (widened to fp16 on host), [.., 2g] bias
    cpk = nc.dram_tensor("cpk", [128, n_ot, 2 * g + 1], F16, kind="ExternalInput").ap()
    outT = nc.dram_tensor("outT", [oc, m], F16, kind="ExternalOutput").ap()

    # dequant engine split per group index: 5 DVE / 3 ACT out of 8
    # (GPSIMD intentionally unused: it port-muxes with DVE and is ~7x slower)
    deq_pattern = [0, 1, 0, 0, 1, 0, 0, 1]

    with tile.TileContext(nc) as tc:
        with (
            tc.tile_pool(name="x", bufs=1) as xpool,
            tc.tile_pool(name="consts", bufs=1) as cpool,
            tc.tile_pool(name="warm", bufs=1) as warmpool,
            tc.tile_pool(name="qw", bufs=3) as qwpool,
            tc.tile_pool(name="wd", bufs=10) as wdpool,
            tc.tile_pool(name="w", bufs=21) as wpool,
            tc.tile_pool(name="ps", bufs=8, space="PSUM") as pspool,
            tc.tile_pool(name="o", bufs=4) as opool,
        ):
            # resident transposed activations: [128, g, m]
            xT_sb = xpool.tile([128, g, m], F16)

            def warmup():
                # lift the HAM clock gate to K=8/8 while the prologue DMAs
                # land: ~12 cold matmuls on a zeroed scratch tile keep the PE
                # busy >3.4us, so the real stream starts at full clock.
                wsrc = warmpool.tile([128, MM_N], F16, name="warm_src")
                wps = pspool.tile([128, MM_N], F32, name="warm_ps", tag="ps")
                nc.gpsimd.memset(wsrc[:], 0.0)
                for _ in range(WARMUP_MMS):
                    nc.tensor.matmul(
                        wps[:], wsrc[:, :128], wsrc[:], start=True, stop=True
                    )

            def load_consts_dma():
                c_t = cpool.tile([128, n_ot, 2 * g + 1], F16, tag="cpk")
                nc.sync.dma_start(c_t[:], cpk[:])
                return c_t

            def consts_compute(c_t):
                s_t = cpool.tile([128, n_ot, g], F32, tag="s")
                zs_t = cpool.tile([128, n_ot, g], F32, tag="zs")
                nzs_t = cpool.tile([128, n_ot, g], F32, tag="nzs")
                s16 = c_t[:, :, 0:g]
                z16 = c_t[:, :, g : 2 * g]
                b16 = c_t[:, :, 2 * g : 2 * g + 1]
                nc.vector.tensor_copy(s_t[:], s16)
                nc.vector.tensor_tensor(
                    zs_t[:], z16, s16, mybir.AluOpType.mult
                )
                # on ACT: also warms the activation table before dequant needs it
                nc.scalar.activation(
                    nzs_t[:], zs_t[:], mybir.ActivationFunctionType.Identity,
                    scale=-1.0,
                )
                return s_t, zs_t, nzs_t, b16

            def load_qw_triple():
                # otiles 0-2 in one DMA: 12KB-contiguous per-partition rows
                qw_t = qwpool.tile([128, 3, k], U8, name="qw_012", tag="qw3", bufs=1)
                nc.scalar.dma_start(qw_t[:], qwP[:, 0:3])
                return qw_t[:, 0], qw_t[:, 1], qw_t[:, 2]

            def load_qw_pair(ot, eng=None):
                # two-otile load: one DMA, 8KB-contiguous per-partition rows
                qw_t = qwpool.tile(
                    [128, 2, k], U8, name=f"qw_{ot}_{ot+1}", tag="qw2", bufs=2
                )
                (eng or nc.scalar).dma_start(qw_t[:], qwP[:, ot : ot + 2])
                return qw_t[:, 0], qw_t[:, 1]

            def load_x(g0, sizes, eng):
                for sz in sizes:
                    eng.dma_start(xT_sb[:, g0 : g0 + sz], x3[:, g0 : g0 + sz])
                    g0 += sz
                return g0

            def prep_chunk(ot, qw_ap, consts, q, w3_lists):
                """Dequant + transpose chunk q (QG groups) of otile ot."""
                o0 = ot * 128
                ob = min(128, oc - o0)
                s_all, zs_all, nzs_all, _ = consts
                s_t = s_all[:ob, ot]
                zs_t = zs_all[:ob, ot]
                nzs_t = nzs_all[:ob, ot]
                gbase = q * QG
                wd_t = wdpool.tile([ob, QG * GS], F16, tag="wd")
                w3_t = wpool.tile([128, QG, ob], F16, tag="w")
                for j in range(QG):
                    gi = gbase + j
                    dst = wd_t[:, j * GS : (j + 1) * GS]
                    src = qw_ap[:ob, gi * GS : (gi + 1) * GS]
                    if deq_pattern[gi % len(deq_pattern)]:
                        nc.scalar.activation(
                            dst,
                            src,
                            mybir.ActivationFunctionType.Identity,
                            bias=nzs_t[:, gi : gi + 1],
                            scale=s_t[:, gi : gi + 1],
                        )
                    else:
                        nc.vector.tensor_scalar(
                            dst,
                            src,
                            s_t[:, gi : gi + 1],
                            zs_t[:, gi : gi + 1],
                            mybir.AluOpType.mult,
                            mybir.AluOpType.subtract,
                        )
                nc.sync.dma_start_transpose(w3_t[:], wd_t[:])
                w3_lists.setdefault(ot, {})[q] = w3_t
                return wd_t

            def emit_prep(ot, qw_ap, consts, w3_lists):
                for q in range(g // QG):
                    prep_chunk(ot, qw_ap, consts, q, w3_lists)
                o0 = ot * 128
                ob = min(128, oc - o0)
                return (o0, ob, w3_lists[ot])

            mslices = [
                slice(mc * MM_N, min(m, (mc + 1) * MM_N)) for mc in range(n_mch)
            ]

            def make_pss(ot, ob):
                return [
                    pspool.tile([ob, MM_N], F32, name=f"ps_{ot}_{mc}", tag="ps")
                    for mc in range(n_mch)
                ]

            def mm_one(prep, pss, gi, mc):
                o0, ob, w3_list = prep
                msl = mslices[mc]
                nc.tensor.matmul(
                    pss[mc][:, : msl.stop - msl.start],
                    w3_list[gi // QG][:, gi % QG, :],
                    xT_sb[:, gi, msl],
                    start=(gi == 0),
                    stop=(gi == g - 1),
                )

            def evac(prep, pss, consts, mcs=None):
                o0, ob, _ = prep
                b16 = consts[3]
                b_t = b16[:ob, o0 // 128]
                for mc in mcs if mcs is not None else range(n_mch):
                    msl = mslices[mc]
                    mn = msl.stop - msl.start
                    o_t = opool.tile([ob, MM_N], F16, name="o_t", tag="o")
                    nc.scalar.activation(
                        o_t[:, :mn],
                        pss[mc][:, :mn],
                        mybir.ActivationFunctionType.Identity,
                        bias=b_t[:],
                        scale=1.0,
                    )
                    nc.gpsimd.dma_start(outT[o0 : o0 + ob, msl], o_t[:, :mn])

            # ---- emission ----
            # ring assignment for the front: sync carries the packed consts,
            # otile 0's weights and ALL transposes (nothing slow ahead of
            # them); the scalar HWDGE ring carries the otile 1/2 pair load,
            # the x tail, and all steady qw pairs; the gpsimd SWDGE ring
            # carries the x head. Three rings drain concurrently and no
            # early DMA waits on an unrelated one.
            warmup()
            c_t = load_consts_dma()
            qws = {}
            qws[0], qws[1], qws[2] = load_qw_triple()  # scalar, first in FIFO
            gx = load_x(0, X_SC, nc.scalar)
            load_x(gx, X_GP, nc.gpsimd)
            consts = consts_compute(c_t)
            # dequant otiles 0/1/2 interleaved at chunk granularity so each
            # otile's first groups are transposed ~when phase 1 needs them
            w3_lists = {}
            preps = {}
            deq_order = [
                (0, 0), (0, 1), (1, 0), (2, 0),
                (0, 2), (1, 1), (2, 1),
                (0, 3), (1, 2), (2, 2),
                (1, 3), (2, 3),
            ]
            for ot, q in deq_order:
                prep_chunk(ot, qws[ot], consts, q, w3_lists)
            for ot in range(3):
                preps[ot] = (ot * 128, 128, w3_lists[ot])

            # phase 1: x-chunk-aligned bursts over otiles 0-2 — otile 0 rides
            # the x frontier; the otile-1/2 bursts replay the just-landed
            # groups, covering the wait for the next 8-group x chunk
            PHI = 3
            pss_phi = {ot: make_pss(ot, preps[ot][1]) for ot in range(PHI)}
            for blk in range(g // QG):
                for ot in range(PHI):
                    for gi in range(blk * QG, (blk + 1) * QG):
                        for mc in range(n_mch):
                            mm_one(preps[ot], pss_phi[ot], gi, mc)

            # prefetch the first steady otiles while phase 1 streams
            qws[3], qws[4] = load_qw_pair(3)
            preps[3] = emit_prep(3, qws[3], consts, w3_lists)
            preps[4] = emit_prep(4, qws[4], consts, w3_lists)
            for ot in range(PHI):
                evac(preps[ot], pss_phi[ot], consts)

            # steady phase: otiles 3..n_otiles-1 sequential, mc-interleaved
            AHEAD = 2
            for ot in range(PHI, n_otiles):
                nxt = ot + AHEAD
                if nxt < n_otiles and nxt >= 5:
                    if nxt + 1 < n_otiles and nxt % 2 == 1:
                        qws[nxt], qws[nxt + 1] = load_qw_pair(nxt)
                    assert nxt in qws
                    preps[nxt] = emit_prep(nxt, qws[nxt], consts, w3_lists)
                pss = make_pss(ot, preps[ot][1])
                if ot == n_otiles - 1:
                    # chain-major: mc0's ACT+store overlap mc1's matmuls,
                    # shrinking the exposed tail after the last matmul
                    for mc in range(n_mch):
                        for gi in range(g):
                            mm_one(preps[ot], pss, gi, mc)
                        evac(preps[ot], pss, consts, mcs=[mc])
                else:
                    for gi in range(g):
                        for mc in range(n_mch):
                            mm_one(preps[ot], pss, gi, mc)
                    evac(preps[ot], pss, consts)

    nc.compile()
    return nc


def _get_nc(m=M, k=IN, oc=OC, n_cores=N_CORES):
    key = (m, k, oc, n_cores)
    if key not in _CACHE:
        _CACHE[key] = _build(*key)
    return _CACHE[key]


def _make_in_maps(x, qweight, qzeros, scales, bias, n_cores=N_CORES):
    iw8 = _unpack_int4(qweight)  # [IN, OUT] uint8
    iz8 = _unpack_int4(qzeros)  # [G, OUT] uint8
    kk, mm = x.shape[1], x.shape[0]
    # [p, group, m]: partition-major so each partition's slab is contiguous
    x3 = np.ascontiguousarray(x.T.reshape(kk // GS, GS, mm).transpose(1, 0, 2))
    oc = qweight.shape[1] * PACK // n_cores
    n_ot = (oc + 127) // 128
    ocp = n_ot * 128

    def pm(a):
        # pad rows to whole otiles, then [ocp, d] -> [128, n_ot, d]
        a = np.pad(a, [(0, ocp - oc)] + [(0, 0)] * (a.ndim - 1))
        return np.ascontiguousarray(a.reshape(n_ot, 128, -1).transpose(1, 0, 2))

    in_maps = []
    for c in range(n_cores):
        sl = slice(c * oc, (c + 1) * oc)
        # pack scales, zeros (widened to fp16 — layout-only), bias into one
        # fp16 tensor: [oc, 2g+1]
        cpk = np.concatenate(
            [
                scales[:, sl].T.astype(np.float16),
                iz8[:, sl].T.astype(np.float16),
                bias[sl].reshape(-1, 1).astype(np.float16),
            ],
            axis=1,
        )
        in_maps.append(
            {
                "x3": x3,
                "qwP": pm(iw8[:, sl].T),
                "cpk": pm(cpk),
            }
        )
    return in_maps


LAST_EXEC_NS = None


def kernel(x, qweight, qzeros, scales, bias):
    global LAST_EXEC_NS
    from concourse.bass_utils import run_bass_kernel_spmd

    x = np.asarray(x)
    qweight = np.asarray(qweight)
    qzeros = np.asarray(qzeros)
    scales = np.asarray(scales)
    bias = np.asarray(bias)

    nc = _get_nc()
    in_maps = _make_in_maps(x, qweight, qzeros, scales, bias)

    kwargs = {}
    if os.environ.get("AWQ_PROFILE"):
        _enable_profiling()
        kwargs = dict(trace=True, tmpdir=os.environ.get("AWQ_TRACE_DIR") or None)
    res = run_bass_kernel_spmd(nc, in_maps, list(range(N_CORES)), **kwargs)
    LAST_EXEC_NS = res.exec_time_ns

    outT = np.concatenate([res.results[c]["outT"] for c in range(N_CORES)], axis=0)
    return np.ascontiguousarray(outT.T)


def _enable_profiling():
    """Register the NTFF profile hook missing from this image's antenv."""
    import types

    if "antenv.axon_hooks" not in sys.modules:
        import antenv

        mod = types.ModuleType("antenv.axon_hooks")
        mod._hook = None
        mod.set_axon_ntff_profile_hook = lambda h: setattr(mod, "_hook", h)
        mod.get_axon_ntff_profile_hook = lambda: mod._hook
        sys.modules["antenv.axon_hooks"] = mod
        antenv.axon_hooks = mod
        try:
            from trn_agent_boot.trn_boot import _ntff_profile_via_ctypes

            mod.set_axon_ntff_profile_hook(
                _ntff_profile_via_ctypes("/opt/axon/libaxon_pjrt.so")
            )
        except Exception:
            pass
    import concourse.bass_utils as _bu

    _bu.upload_artifacts = lambda tmpdir: "local://skipped"

